# revision 1
# baseline (speedup 1.0000x reference)
# Trainium2 Bass kernel for the LeNet-C3 sparse-connection conv problem.
#
# Math: VALID 2D conv, input [32, 512, 512, 6] f32, dense kernel [5,5,6,16]
# (assembled from the sparse C3 connection tables), + bias -> [32, 508, 508, 16].
#
# Strategy (pure data parallel, 4 images per core x 8 cores):
#   - Host: assemble dense weights, build per-filter-row "width-unrolled"
#     stationary matrices W_big[dy] of shape [K=72, M=128]:
#       K = 12 x-positions x 6 channels (flattened (x,c), matching the DRAM
#           channel-last row layout), M = 8 output pixels x 16 out channels.
#   - Device: for each group of 8 output columns, 5 accumulating matmuls
#     (one per filter row dy) with N = 508 output rows as the moving free dim:
#       psum[128, 508] += W_big[dy].T @ x_rows[72, 508]
#     then bias-add (ScalarE/VectorE alternating) and DMA out.
#   - Inputs are cast to bf16 on host (PSUM accumulates in f32); outputs f32.
#
# Per image the flat row is 512*6 = 3072 values, padded to 3096 so the last
# 8-pixel group (x=504..511, only 504..507 valid) can read a full 72-wide
# window. Group g reads flat columns [48g, 48g+72).

import numpy as np
import ml_dtypes

BATCH, H, W, CIN, COUT, FS = 32, 512, 512, 6, 16, 5
N_CORES = 8
IMGS_PER_CORE = BATCH // N_CORES  # 4
HO = WO = H - FS + 1  # 508
FLAT = W * CIN  # 3072
FLAT_PAD = 3096  # 48*63 + 72 = 3096 (covers last group's window)
GROUPS = 64  # ceil(508/8) groups of 8 output columns
KDIM = 72  # 12 x-positions * 6 channels
MDIM = 128  # 8 pixels * 16 out channels
GCHUNK = 8  # groups per SBUF input chunk (DMA/compute pipelining granularity)

_CACHE = {}


def _dense_kernel_np(weights3, weights4, weights4_4, weights6):
    """Numpy port of reference._dense_kernel: [5,5,6,16] dense conv kernel."""
    f = weights3.shape[0]
    Wd = np.zeros((f, f, CIN, COUT), dtype=np.float32)
    for i in range(6):
        for m in range(3):
            Wd[:, :, (i + m) % 6, i] = weights3[:, :, m, i]
    for k in range(6):
        for m in range(4):
            Wd[:, :, (k + m) % 6, 6 + k] = weights4[:, :, m, k]
    for k in range(3):
        for m, off in enumerate((0, 1, 3, 4)):
            Wd[:, :, (k + off) % 6, 12 + k] = weights4_4[:, :, m, k]
    Wd[:, :, :, 15] = weights6[:, :, :, 0]
    return Wd


def _build_wbig(Wd):
    """[FS, KDIM, MDIM]: W_big[dy, r*6+c, j*16+co] = Wd[dy, r-j, c, co]."""
    wb = np.zeros((FS, KDIM, MDIM), dtype=np.float32)
    for dy in range(FS):
        for r in range(12):
            for j in range(8):
                dx = r - j
                if 0 <= dx < FS:
                    wb[dy, r * 6:(r + 1) * 6, j * 16:(j + 1) * 16] = Wd[dy, dx]
    return wb


def _split_excess_waits(nc, max_waits=1):
    """This image's walrus rejects instructions carrying more than one sem
    wait ("Too many sync wait commands" in setupSyncWait). Tile freely
    attaches several waits to one instruction. Hoist the extras onto
    nofuse NOPs inserted just before, on the same engine — identical
    semantics (all waits retired before the instruction issues)."""
    import concourse.mybir as mybir

    for f in nc.m.functions:
        for bb in f.blocks:
            new_list = []
            changed = False
            for inst in bb.instructions:
                si = inst.sync_info
                waits = list(si.on_wait) if si and si.on_wait else []
                if len(waits) > max_waits:
                    changed = True
                    for k, w in enumerate(waits[max_waits:]):
                        nop = mybir.InstNoOp(
                            name=f"{inst.name}-wsplit{k}",
                            sync_info=mybir.SyncInfo(on_wait=[w], on_update=[]),
                            bass_nofuse=True,
                            engine=inst.engine,
                        )
                        new_list.append(nop)
                    si.on_wait = waits[:max_waits]
                new_list.append(inst)
            if changed:
                bb.instructions = new_list


def _build_nc(n_imgs=IMGS_PER_CORE):
    import concourse.bass as bass
    import concourse.mybir as mybir
    from concourse.tile import TileContext

    nc = bass.Bass(trn_type="TRN2")
    x = nc.dram_tensor("x", (n_imgs, H, FLAT_PAD), mybir.dt.bfloat16,
                       kind="ExternalInput")
    w = nc.dram_tensor("w", (KDIM, FS * MDIM), mybir.dt.bfloat16,
                       kind="ExternalInput")
    b = nc.dram_tensor("b", (MDIM, 1), mybir.dt.float32, kind="ExternalInput")
    out = nc.dram_tensor("out", (n_imgs, HO, WO, COUT), mybir.dt.float32,
                         kind="ExternalOutput")

    with TileContext(nc) as tc:
        with tc.tile_pool(name="const", bufs=1) as cpool, \
             tc.tile_pool(name="xin", bufs=16) as xpool, \
             tc.tile_pool(name="stage", bufs=6) as spool, \
             tc.tile_pool(name="ps", bufs=8, space="PSUM") as ppool:
            wt = cpool.tile([KDIM, FS * MDIM], mybir.dt.bfloat16, name="wt")
            nc.sync.dma_start(out=wt[:, :], in_=w[:, :])
            bt = cpool.tile([MDIM, 1], mybir.dt.float32, name="bt")
            nc.sync.dma_start(out=bt[:, :], in_=b[:, :])

            for n in range(n_imgs):
                for g in range(GROUPS):
                    # [72 partitions, 512 rows]: window of 12 x-positions
                    # (x,c)-flat, contiguous 72 elems at flat offset 48g.
                    xt = xpool.tile([KDIM, H], mybir.dt.bfloat16,
                                    name="xt", tag="xt")
                    nc.sync.dma_start(
                        out=xt[:, :],
                        in_=x[n, :, 48 * g:48 * g + KDIM].rearrange("y f -> f y"),
                    )
                    ps = ppool.tile([MDIM, HO], mybir.dt.float32,
                                    name="ps", tag="ps")
                    for dy in range(FS):
                        nc.tensor.matmul(
                            ps[:, :],
                            wt[:, dy * MDIM:(dy + 1) * MDIM],
                            xt[:, dy: dy + HO],
                            start=(dy == 0), stop=(dy == FS - 1),
                        )
                    st = spool.tile([MDIM, HO], mybir.dt.float32,
                                    name="st", tag="st")
                    if g % 2 == 0:
                        nc.scalar.activation(
                            st[:, :], ps[:, :],
                            mybir.ActivationFunctionType.Identity, bias=bt[:, :])
                    else:
                        nc.vector.tensor_scalar_add(st[:, :], ps[:, :], bt[:, :])
                    m = MDIM if g < GROUPS - 1 else 64  # last group: 4 px valid
                    nc.sync.dma_start(
                        out=out[n, :, 8 * g:8 * g + m // COUT, :]
                            .rearrange("y x c -> (x c) y"),
                        in_=st[0:m, :],
                    )
    _split_excess_waits(nc)
    return nc


def _prep_shared(weights3, weights4, weights4_4, weights6, bias1):
    Wd = _dense_kernel_np(np.asarray(weights3, np.float32),
                          np.asarray(weights4, np.float32),
                          np.asarray(weights4_4, np.float32),
                          np.asarray(weights6, np.float32))
    wb = _build_wbig(Wd)  # [5, 72, 128]
    w_flat = np.ascontiguousarray(
        wb.transpose(1, 0, 2).reshape(KDIM, FS * MDIM)).astype(ml_dtypes.bfloat16)
    b_vec = np.ascontiguousarray(
        np.tile(np.asarray(bias1, np.float32), 8)[:, None])
    return w_flat, b_vec


def run(inputs, weights3, weights4, weights4_4, weights6, bias1, trace=False):
    from concourse.bass_utils import run_bass_kernel_spmd

    if "nc" not in _CACHE:
        _CACHE["nc"] = _build_nc()
    nc = _CACHE["nc"]

    w_flat, b_vec = _prep_shared(weights3, weights4, weights4_4, weights6, bias1)

    xin = np.asarray(inputs, np.float32).reshape(BATCH, H, FLAT)
    xpad = np.zeros((BATCH, H, FLAT_PAD), dtype=ml_dtypes.bfloat16)
    xpad[:, :, :FLAT] = xin.astype(ml_dtypes.bfloat16)

    in_maps = [
        {"x": xpad[c * IMGS_PER_CORE:(c + 1) * IMGS_PER_CORE],
         "w": w_flat, "b": b_vec}
        for c in range(N_CORES)
    ]
    res = run_bass_kernel_spmd(nc, in_maps, core_ids=list(range(N_CORES)),
                               trace=trace)
    out = np.concatenate([r["out"] for r in res.results], axis=0)
    return out, res


def kernel(inputs, weights3, weights4, weights4_4, weights6, bias1):
    out, _ = run(inputs, weights3, weights4, weights4_4, weights6, bias1)
    return out

